# revision 1
# baseline (speedup 1.0000x reference)
"""CAAN attention-scorer kernel for 8 Trainium2 NeuronCores.

scores = relu(softmax(QK^T/sqrt(D)) @ V @ W1 + b1) @ W2 + b2
with Q/K/V = x @ W{q,k,v} + b{q,k,v};  N=8192, IN_DIM=1024, D=512.

Row-parallel attention: core c owns rows [c*1024, (c+1)*1024). K/V are
computed (replicated) on every core from the full x^T — with fp8 DoubleRow
matmuls (2x PE rate) the replicated projections are cheaper than an
AllGather in this environment (~90us/call collective floor).

Everything runs in a transposed layout so softmax denominators come from a
constant-column PE matmul and normalization is deferred into the ctx PSUM
evacuation:

  kT[d, j]   = (8 Wk)^T x^T (+8 bk)   d on partitions   (fp8, x8 scale)
  qT[d, i]   = (8 Wq)^T xq^T (+8 bq)                    (fp8, x8 scale)
  v[j, d]    = x (8 Wv) (+8 bv)       j on partitions   (fp8 DRAM scratch)
  S^T[j, i]  = kT-chunk^T qT          (= 64 * true scores)
  E          = exp(S^T / (64 sqrt(D)))     PSUM -> SBUF fp8
  ctxU^T    += v-chunk^T E            (= 8 * true ctxU)
  den[1,i]  += (8.0)^T E              (= 8 * true denom -> cancels the x8)
  ctx^T      = ctxU^T * (1/den)       broadcast via ones (x) recip matmul
  h^T[m, i]  = relu(W1-chunk^T ctx^T + b1)    (bf16 MLP)
  out[1, i]  = W2^T h^T + b2

The x8 weight pre-scaling (exact power of two) keeps the uniform(+-1/32)
weights out of fp8-e4m3 denormal range. Measured end-to-end max-rel-err vs
the f32 reference: ~9e-3 (host simulation).
"""

import numpy as np
import ml_dtypes

import concourse.tile as tile
from concourse import bacc, mybir
from concourse.bass_utils import run_bass_kernel_spmd

N, IN_DIM, D = 8192, 1024, 512
NCORES = 8
NB = N // NCORES            # 1024 rows per core
P = 128
KC = IN_DIM // P            # 8  k-chunks of the input dim
KP = KC // 2                # 4  DoubleRow k-pairs
DC = D // P                 # 4  d-chunks
DP = DC // 2                # 2  DoubleRow d-pairs
MC = (D // 2) // P          # 2  m-chunks of the hidden dim
JT = 512                    # j-tile width in phase 1
NJT = N // JT               # 16
NJC = N // P                # 64 j-chunks
NJP = NJC // 2              # 32 j-pairs in phase 2
IH = 512                    # i-half width in phase 2
NIH = NB // IH              # 2
WS = 8.0                    # fp8 weight pre-scale (exact power of two)
SCALE = 1.0 / float(np.sqrt(np.float32(D))) / (WS * WS)

FP8 = mybir.dt.float8e4
BF16 = mybir.dt.bfloat16
F32 = mybir.dt.float32
DR = mybir.MatmulPerfMode.DoubleRow

_CACHE = {}


def _build():
    nc = bacc.Bacc(None, target_bir_lowering=False, debug=False)

    xt = nc.declare_dram_parameter("xt", [P, KC, N], FP8, isOutput=False)
    xq = nc.declare_dram_parameter("xq", [P, KC, NB], FP8, isOutput=False)
    wq = nc.declare_dram_parameter("wq", [P, KC, D], FP8, isOutput=False)
    wk = nc.declare_dram_parameter("wk", [P, KC, D], FP8, isOutput=False)
    wv = nc.declare_dram_parameter("wv", [P, KC, D], FP8, isOutput=False)
    w1 = nc.declare_dram_parameter("w1", [P, DC, MC, P], BF16, isOutput=False)
    w2 = nc.declare_dram_parameter("w2", [P, MC], BF16, isOutput=False)
    bq = nc.declare_dram_parameter("bq", [P, DC], F32, isOutput=False)   # x8
    bk = nc.declare_dram_parameter("bk", [P, DC], F32, isOutput=False)   # x8
    bv = nc.declare_dram_parameter("bv", [P, D], F32, isOutput=False)    # x8
    b1 = nc.declare_dram_parameter("b1", [P, MC], F32, isOutput=False)
    b2 = nc.declare_dram_parameter("b2", [1, 1], F32, isOutput=False)
    out = nc.declare_dram_parameter("out", [1, NB], F32, isOutput=True)

    with tile.TileContext(nc) as tc:
        with (
            tc.tile_pool(name="singles", bufs=1) as singles,
            tc.tile_pool(name="dram", bufs=1, space="DRAM") as dram,
        ):
            # ---- constants / weights into SBUF ----
            wq_sb = singles.tile([P, KC, D], FP8)
            wk_sb = singles.tile([P, KC, D], FP8)
            wv_sb = singles.tile([P, KC, D], FP8)
            w1_sb = singles.tile([P, DC, MC, P], BF16)
            w2_sb = singles.tile([P, MC], BF16)
            bq_sb = singles.tile([P, DC], F32)
            bk_sb = singles.tile([P, DC], F32)
            bv_sb = singles.tile([P, D], F32)
            b1_sb = singles.tile([P, MC], F32)
            b2_sb = singles.tile([1, 1], F32)
            cs_w = singles.tile([P, 2, 32], FP8)     # colsum weights = 8.0
            # (dual-fp8 ldweights needs >=32 active columns; rows identical)
            ones_f32 = singles.tile([1, P], F32)
            for kp in range(KP):
                nc.sync.dma_start(wq_sb[:, 2 * kp:2 * kp + 2],
                                  wq[:, 2 * kp:2 * kp + 2])
            nc.sync.dma_start(bq_sb[:], bq[:])
            for dst, src in [(wk_sb, wk), (bk_sb, bk), (wv_sb, wv), (bv_sb, bv),
                             (w1_sb, w1), (w2_sb, w2), (b1_sb, b1), (b2_sb, b2)]:
                nc.gpsimd.dma_start(out=dst[:], in_=src[:])
            nc.vector.memset(cs_w[:], WS)
            nc.vector.memset(ones_f32[:], 1.0)

            # persistent activations
            kt_sb = singles.tile([P, DC, N], FP8)       # kT, d on partitions
            qt_sb = singles.tile([P, DC, NB], FP8)      # qT
            v_sb = singles.tile([P, NJC, D], FP8)       # v, j on partitions

            # ---- all pools at top level: one fungible [128,512] PSUM tag so
            # phase-1 projection and phase-2 attention matmuls can interleave
            with (
                tc.tile_pool(name="xtiles", bufs=3) as xtiles,
                tc.tile_pool(name="evac", bufs=6) as evac,
                tc.tile_pool(name="etile", bufs=6) as etile,
                tc.tile_pool(name="mlp", bufs=2) as mlp,
                tc.tile_pool(name="ps_mm", bufs=3, space="PSUM") as ps_mm,
                tc.tile_pool(name="ps_ctx", bufs=1, space="PSUM") as ps_ctx,
                tc.tile_pool(name="ps_cs", bufs=1, space="PSUM") as ps_cs,
            ):
                # qT first: phase 2's S_T depends on it, so emitting it early
                # lets attention matmuls start as soon as kT tiles land
                for it in range(NB // JT):
                    xq_t = xtiles.tile([P, KC, JT], FP8, tag="xt")
                    if it == 0:
                        for kp in range(KP):
                            nc.sync.dma_start(
                                xq_t[:, 2 * kp:2 * kp + 2],
                                xq[:, 2 * kp:2 * kp + 2, it * JT:(it + 1) * JT])
                    else:
                        nc.sync.dma_start(xq_t[:],
                                          xq[:, :, it * JT:(it + 1) * JT])
                    for dc in range(DC):
                        ps = ps_mm.tile([P, JT], F32, tag="st")
                        for kp in range(KP):
                            nc.tensor.matmul(
                                ps[:],
                                wq_sb[:, 2 * kp:2 * kp + 2, dc * P:(dc + 1) * P],
                                xq_t[:, 2 * kp:2 * kp + 2],
                                start=(kp == 0), stop=(kp == KP - 1),
                                perf_mode=DR)
                        nc.vector.tensor_scalar_add(
                            qt_sb[:, dc, it * JT:(it + 1) * JT], ps[:],
                            bq_sb[:, dc:dc + 1])

                for jt in range(NJT):
                    xt_t = xtiles.tile([P, KC, JT], FP8, tag="xt")
                    if jt == 0:
                        for kp in range(KP):
                            nc.sync.dma_start(
                                xt_t[:, 2 * kp:2 * kp + 2],
                                xt[:, 2 * kp:2 * kp + 2, jt * JT:(jt + 1) * JT])
                    else:
                        nc.sync.dma_start(xt_t[:],
                                          xt[:, :, jt * JT:(jt + 1) * JT])
                    # kT tile: [P(d), JT] per d-chunk
                    for dc in range(DC):
                        ps = ps_mm.tile([P, JT], F32, tag="st")
                        for kp in range(KP):
                            nc.tensor.matmul(
                                ps[:],
                                wk_sb[:, 2 * kp:2 * kp + 2, dc * P:(dc + 1) * P],
                                xt_t[:, 2 * kp:2 * kp + 2],
                                start=(kp == 0), stop=(kp == KP - 1),
                                perf_mode=DR)
                        nc.vector.tensor_scalar_add(
                            kt_sb[:, dc, jt * JT:(jt + 1) * JT], ps[:],
                            bk_sb[:, dc:dc + 1])
                    # v chunks: [P(j), D] -> DRAM scratch
                    for jc in range(JT // P):
                        ps = ps_mm.tile([P, D], F32, tag="st")
                        for kp in range(KP):
                            nc.tensor.matmul(
                                ps[:],
                                xt_t[:, 2 * kp:2 * kp + 2, jc * P:(jc + 1) * P],
                                wv_sb[:, 2 * kp:2 * kp + 2],
                                start=(kp == 0), stop=(kp == KP - 1),
                                perf_mode=DR)
                        nc.vector.tensor_tensor(
                            v_sb[:, jt * (JT // P) + jc, :], ps[:], bv_sb[:],
                            mybir.AluOpType.add)

            # ---- phase 2: attention + MLP per i-half ----
                out_sb = singles.tile([1, NB], F32)
                for ih in range(NIH):
                    i0 = ih * IH
                    ctx_ps = ps_ctx.tile([P, DC, IH], F32)
                    cs_ps = ps_cs.tile([32, IH], F32)
                    for t in range(NJP):
                        e_t = etile.tile([P, 2, IH], FP8, tag="et")
                        for s in range(2):
                            jc = 2 * t + s
                            st_ps = ps_mm.tile([P, IH], F32, tag="st")
                            for dp in range(DP):
                                nc.tensor.matmul(
                                    st_ps[:],
                                    kt_sb[:, 2 * dp:2 * dp + 2,
                                          jc * P:(jc + 1) * P],
                                    qt_sb[:, 2 * dp:2 * dp + 2, i0:i0 + IH],
                                    start=(dp == 0), stop=(dp == DP - 1),
                                    perf_mode=DR)
                            nc.scalar.activation(
                                e_t[:, s], st_ps[:],
                                mybir.ActivationFunctionType.Exp,
                                bias=0.0, scale=SCALE)
                        nc.tensor.matmul(cs_ps[:], cs_w[:], e_t[:],
                                         start=(t == 0), stop=(t == NJP - 1),
                                         perf_mode=DR)
                        for dc in range(DC):
                            nc.tensor.matmul(
                                ctx_ps[:, dc],
                                v_sb[:, 2 * t:2 * t + 2, dc * P:(dc + 1) * P],
                                e_t[:],
                                start=(t == 0), stop=(t == NJP - 1),
                                perf_mode=DR)

                    # softmax denominators -> broadcast reciprocal
                    recip_sb = mlp.tile([1, IH], F32, tag="recip")
                    nc.vector.reciprocal(recip_sb[:], cs_ps[0:1])
                    r_ps = ps_mm.tile([P, IH], F32, tag="st")
                    nc.tensor.matmul(r_ps[:], ones_f32[:], recip_sb[:],
                                     start=True, stop=True)
                    r_sb = mlp.tile([P, IH], F32, tag="rsb")
                    nc.vector.tensor_copy(r_sb[:], r_ps[:])

                    # normalized ctx^T (bf16) at PSUM evacuation
                    ctxn = mlp.tile([P, DC, IH], BF16, tag="ctxn")
                    for dc in range(DC):
                        nc.vector.tensor_tensor(ctxn[:, dc], ctx_ps[:, dc],
                                                r_sb[:], mybir.AluOpType.mult)

                    # h^T = relu(W1-chunk^T ctx^T + b1); out = W2^T h^T + b2
                    sc_ps = ps_mm.tile([1, IH], F32, tag="st")
                    h_sb = mlp.tile([P, MC, IH], BF16, tag="hsb")
                    for mc in range(MC):
                        g_ps = ps_mm.tile([P, IH], F32, tag="st")
                        for dc in range(DC):
                            nc.tensor.matmul(g_ps[:], w1_sb[:, dc, mc],
                                             ctxn[:, dc],
                                             start=(dc == 0), stop=(dc == DC - 1))
                        nc.scalar.activation(h_sb[:, mc], g_ps[:],
                                             mybir.ActivationFunctionType.Relu,
                                             bias=b1_sb[:, mc:mc + 1], scale=1.0)
                    for mc in range(MC):
                        nc.tensor.matmul(sc_ps[:], w2_sb[:, mc:mc + 1],
                                         h_sb[:, mc],
                                         start=(mc == 0), stop=(mc == MC - 1))
                    nc.scalar.add(out_sb[:, i0:i0 + IH], sc_ps[:], b2_sb[:])

            nc.sync.dma_start(out[:], out_sb[:])

    nc.finalize()
    return nc


def _prep(inputs):
    """Host-side layout prep shared by all cores + per-core xq blocks."""
    f32 = np.float32
    bf16 = ml_dtypes.bfloat16
    fp8 = ml_dtypes.float8_e4m3
    x = np.ascontiguousarray(inputs["x"], dtype=f32)
    xt = np.ascontiguousarray(x.T)                                   # [IN, N]
    xt_r = np.ascontiguousarray(
        xt.reshape(KC, P, N).transpose(1, 0, 2).astype(fp8))         # [P, KC, N]

    def w_r(w):  # [IN, D] -> [P, KC, D], x8 scale into fp8 range
        return np.ascontiguousarray(
            (np.asarray(w, f32) * WS).reshape(KC, P, D)
            .transpose(1, 0, 2).astype(fp8))

    w1_r = np.ascontiguousarray(
        np.asarray(inputs["W1"], f32).reshape(DC, P, MC, P)
        .transpose(1, 0, 2, 3).astype(bf16))                         # [P, DC, MC, P]
    w2_r = np.ascontiguousarray(
        np.asarray(inputs["W2"], f32).reshape(MC, P).T.astype(bf16))  # [P, MC]

    def b_col(b, nchunks, scale=1.0):  # [nchunks*P] -> [P, nchunks]
        return np.ascontiguousarray(
            (np.asarray(b, f32) * scale).reshape(nchunks, P).T)

    shared = {
        "xt": xt_r,
        "wq": w_r(inputs["Wq"]),
        "wk": w_r(inputs["Wk"]),
        "wv": w_r(inputs["Wv"]),
        "w1": w1_r,
        "w2": w2_r,
        "bq": b_col(inputs["bq"], DC, WS),
        "bk": b_col(inputs["bk"], DC, WS),
        "bv": np.ascontiguousarray(
            np.broadcast_to(np.asarray(inputs["bv"], f32) * WS, (P, D))),
        "b1": b_col(inputs["b1"], MC),
        "b2": np.asarray(inputs["b2"], f32).reshape(1, 1),
    }
    xqs = [np.ascontiguousarray(xt_r[:, :, c * NB:(c + 1) * NB])
           for c in range(NCORES)]
    return shared, xqs


def kernel(**inputs) -> np.ndarray:
    if "nc" not in _CACHE:
        _CACHE["nc"] = _build()
    nc = _CACHE["nc"]
    shared, xqs = _prep(inputs)
    in_maps = [dict(shared, xq=xqs[c]) for c in range(NCORES)]
    res = run_bass_kernel_spmd(nc, in_maps, core_ids=list(range(NCORES)))
    return np.concatenate([res.results[c]["out"][0] for c in range(NCORES)])



# revision 44
# speedup vs baseline: 2.0348x; 2.0348x over previous
"""CAAN attention-scorer kernel for 8 Trainium2 NeuronCores.

scores = relu(softmax(QK^T/sqrt(D)) @ V @ W1 + b1) @ W2 + b2
with Q/K/V = x @ W{q,k,v} + b{q,k,v};  N=8192, IN_DIM=1024, D=512.

Row-parallel attention: core c owns rows [c*1024, (c+1)*1024). K/V are
computed (replicated) on every core from the full x^T with fp8 DoubleRow
matmuls (2x PE rate).

Two bias identities shrink the dataflow:
  - softmax(S + 1 (bk.q)^T) == softmax(S): the K bias adds a per-query
    constant to every score column, so bk is dropped entirely.
  - softmax(S) @ (V0 + 1 bv^T) == softmax(S) @ V0 + bv: the V bias
    commutes out of attention and folds into the MLP bias on the host
    (b1' = b1 + bv @ W1), so kT/v PSUM evacuations are pure copies.

Stage A is a software pipeline over 16 j-tiles: the PE projects kT/v for
tile jt (paired [128,2,512] PSUM groups, copy-evacuated by the Vector
engine) while computing S^T for tile jt-1 into paired PSUM tiles that the
Activation engine exponentiates straight into an SBUF stash E (fp8, 8MB,
both i-halves). kT lives in a rotating 3-tile pool - attention consumes
it one tile behind projection. Stage B is then a dense PE-only pass:
denominator colsums, ctx accumulation per d-chunk from E and v, both
i-halves' normalize + MLP interleaved so the Vector/Act chains hide
behind ctx matmuls.

  qT[d, i]   = (8 Wq)^T xq^T (+8 bq)   d on partitions  (fp8, x8 scale)
  kT[d, j]   = (8 Wk)^T x^T            (fp8, x8 scale)
  v[j, d]    = x (8 Wv)                j on partitions  (fp8)
  S^T[j, i]  = kT-chunk^T qT           (= 64 * true scores)
  E          = exp(S^T / (64 sqrt(D)))      PSUM -> SBUF fp8
  den[1,i]   = (8.0)^T E               (= 8 * true denom -> cancels x8)
  ctxU^T     = v-chunk^T E             (= 8 * true ctxU)
  ctx^T      = ctxU^T * (1/den)        (1/den partition-broadcast on Pool)
  h^T[m, i]  = relu(W1-chunk^T ctx^T + b1')   (bf16 MLP)
  out[1, i]  = W2^T h^T + b2

The x8 weight pre-scaling (exact power of two) keeps the uniform(+-1/32)
weights out of fp8-e4m3 denormal range.
"""

import numpy as np
import ml_dtypes

import concourse.tile as tile
from concourse import bacc, mybir
from concourse.bass_utils import run_bass_kernel_spmd

N, IN_DIM, D = 8192, 1024, 512
NCORES = 8
NB = N // NCORES            # 1024 rows per core
P = 128
KC = IN_DIM // P            # 8  k-chunks of the input dim
KP = KC // 2                # 4  DoubleRow k-pairs
DC = D // P                 # 4  d-chunks
DP = DC // 2                # 2  DoubleRow d-pairs
MC = (D // 2) // P          # 2  m-chunks of the hidden dim
JT = 512                    # j-tile width
NJT = N // JT               # 16
NJC = N // P                # 64 j-chunks
NJP = NJC // 2              # 32 j-pairs
IH = 512                    # i-half width
NIH = NB // IH              # 2
WS = 8.0                    # fp8 weight pre-scale (exact power of two)
SCALE = 1.0 / float(np.sqrt(np.float32(D))) / (WS * WS)

FP8 = mybir.dt.float8e4
BF16 = mybir.dt.bfloat16
F32 = mybir.dt.float32
DR = mybir.MatmulPerfMode.DoubleRow
EXP = mybir.ActivationFunctionType.Exp

_CACHE = {}


def _build():
    nc = bacc.Bacc(None, target_bir_lowering=False, debug=False)

    xt = nc.declare_dram_parameter("xt", [P, KC, N], FP8, isOutput=False)
    xq = nc.declare_dram_parameter("xq", [P, KC, NB], FP8, isOutput=False)
    wq = nc.declare_dram_parameter("wq", [P, KC, D], FP8, isOutput=False)
    wk = nc.declare_dram_parameter("wk", [P, KC, D], FP8, isOutput=False)
    wv = nc.declare_dram_parameter("wv", [P, KC, D], FP8, isOutput=False)
    w1 = nc.declare_dram_parameter("w1", [P, DC, MC, P], BF16, isOutput=False)
    w2 = nc.declare_dram_parameter("w2", [P, MC], BF16, isOutput=False)
    bq = nc.declare_dram_parameter("bq", [P, DC], F32, isOutput=False)   # x8
    b1 = nc.declare_dram_parameter("b1", [P, MC], F32, isOutput=False)   # b1'
    b2 = nc.declare_dram_parameter("b2", [1, 1], F32, isOutput=False)
    out = nc.declare_dram_parameter("out", [1, NB], F32, isOutput=True)

    with tile.TileContext(nc) as tc:
        with (
            tc.tile_pool(name="singles", bufs=1) as singles,
        ):
            # ---- weights / constants ----
            wq_sb = singles.tile([P, KC, D], FP8)
            wk_sb = singles.tile([P, KC, D], FP8)
            wv_sb = singles.tile([P, KC, D], FP8)
            w1_sb = singles.tile([P, DC, MC, P], BF16)
            w2_sb = singles.tile([P, MC], BF16)
            bq_sb = singles.tile([P, DC], F32)
            b1_sb = singles.tile([P, MC], F32)
            b2_sb = singles.tile([1, 1], F32)
            cs_w = singles.tile([P, 2, 32], FP8)     # colsum weights = 8.0
            # (dual-fp8 ldweights needs >=32 active columns; rows identical)
            nc.vector.memset(cs_w[:], WS)

            # persistent activations
            qt_sb = singles.tile([P, DC, NB], FP8)      # qT, d on partitions
            v_sb = singles.tile([P, NJC, D], FP8)       # v, j on partitions
            e_tiles = [singles.tile([P, JT // P, NB], FP8, name=f"e_t{j}")
                       for j in range(NJT)]             # exp'd scores E
            out_sb = singles.tile([1, NB], F32)

            with (
                tc.tile_pool(name="xtiles", bufs=4) as xtiles,
                tc.tile_pool(name="kttiles", bufs=3) as kttiles,
                tc.tile_pool(name="mlp", bufs=2) as mlp,
                tc.tile_pool(name="ps_csp", bufs=1, space="PSUM") as ps_csp,
            ):
                cs_pss = [ps_csp.tile([32, NB // 2], F32, name=f"cs{ih}")
                          for ih in range(NIH)]
                # ---- startup DMAs in explicit critical-path order ----
                H = KC // 2
                xq_t0 = xtiles.tile([P, KC, JT], FP8, tag="xt")
                xt_t0 = xtiles.tile([P, KC, JT], FP8, tag="xt")
                nc.sync.dma_start(wq_sb[:, 0:H], wq[:, 0:H])
                nc.sync.dma_start(xq_t0[:, 0:H], xq[:, 0:H, 0:JT])
                nc.sync.dma_start(wq_sb[:, H:], wq[:, H:])
                nc.sync.dma_start(xq_t0[:, H:], xq[:, H:, 0:JT])
                nc.sync.dma_start(bq_sb[:], bq[:])
                nc.sync.dma_start(wk_sb[:, 0:H], wk[:, 0:H])
                nc.sync.dma_start(xt_t0[:, 0:H], xt[:, 0:H, 0:JT])
                nc.sync.dma_start(wk_sb[:, H:], wk[:, H:])
                nc.sync.dma_start(xt_t0[:, H:], xt[:, H:, 0:JT])

                # ---- qT projection (evac on Act with per-partition bias) --
                def qt_ops(it, xq_t=None):
                    if xq_t is None:
                        xq_t = xtiles.tile([P, KC, JT], FP8, tag="xt")
                        nc.sync.dma_start(xq_t[:],
                                          xq[:, :, it * JT:(it + 1) * JT])
                    if it == 1:
                        nc.sync.dma_start(wv_sb[:, 0:KC // 2], wv[:, 0:KC // 2])
                        nc.sync.dma_start(wv_sb[:, KC // 2:], wv[:, KC // 2:])

                    def qt_pair(dp):
                        ps = ps_mm[0].tile([P, 2, JT], F32, tag="st")
                        for s in range(2):
                            dc = 2 * dp + s
                            for kp in range(KP):
                                nc.tensor.matmul(
                                    ps[:, s],
                                    wq_sb[:, 2 * kp:2 * kp + 2,
                                          dc * P:(dc + 1) * P],
                                    xq_t[:, 2 * kp:2 * kp + 2],
                                    start=(kp == 0), stop=(kp == KP - 1),
                                    perf_mode=DR)
                            nc.scalar.add(
                                qt_sb[:, dc, it * JT:(it + 1) * JT],
                                ps[:, s], bq_sb[:, dc:dc + 1])

                    for dp in range(DP):
                        yield lambda dp=dp: qt_pair(dp)

                ps_mm = []

                kt_tiles = {}

                def proj_ops(jt, xt_t=None):
                    """kT/v projection for tile jt: paired PSUM groups,
                    pure-copy evacuation on the Vector engine."""
                    if xt_t is None:
                        xt_t = xtiles.tile([P, KC, JT], FP8, tag="xt")
                        nc.sync.dma_start(xt_t[:],
                                          xt[:, :, jt * JT:(jt + 1) * JT])
                    if jt == 2:
                        # tail-only weights: queue them after xt2 so they
                        # never delay the startup-critical transfers
                        for dst, src in [(w1_sb, w1), (w2_sb, w2),
                                         (b1_sb, b1), (b2_sb, b2)]:
                            nc.sync.dma_start(dst[:], src[:])
                    kt_t = kttiles.tile([P, DC, JT], FP8, tag="kt")
                    kt_tiles[jt] = kt_t

                    def kt_half(ps, dp, s):
                        dc = 2 * dp + s
                        for kp in range(KP):
                            nc.tensor.matmul(
                                ps[:, s],
                                wk_sb[:, 2 * kp:2 * kp + 2,
                                      dc * P:(dc + 1) * P],
                                xt_t[:, 2 * kp:2 * kp + 2],
                                start=(kp == 0), stop=(kp == KP - 1),
                                perf_mode=DR)

                    def v_half(ps, jp, s):
                        jc = 2 * jp + s
                        for kp in range(KP):
                            nc.tensor.matmul(
                                ps[:, s],
                                xt_t[:, 2 * kp:2 * kp + 2,
                                     jc * P:(jc + 1) * P],
                                wv_sb[:, 2 * kp:2 * kp + 2],
                                start=(kp == 0), stop=(kp == KP - 1),
                                perf_mode=DR)

                    def pair_ops(half, evac_out, idx):
                        ps = ps_mm[0].tile([P, 2, JT], F32, tag="st",
                                           name="proj_ps")
                        yield lambda: half(ps, idx, 0)

                        def second():
                            half(ps, idx, 1)
                            nc.vector.tensor_copy(evac_out, ps[:])
                        yield second

                    for dp in range(DP):
                        yield from pair_ops(kt_half,
                                            kt_t[:, 2 * dp:2 * dp + 2], dp)
                    for jp in range(2):
                        jc0 = jt * (JT // P) + 2 * jp
                        yield from pair_ops(v_half, v_sb[:, jc0:jc0 + 2], jp)

                def attn_ops(jt, ihs=(0, 1)):
                    """S^T + exp for tile jt into the e stash."""
                    kt_t = kt_tiles[jt]

                    def s_half(ps, t, ih, s):
                        jcl = (2 * t + s) % (JT // P)
                        for dp in range(DP):
                            nc.tensor.matmul(
                                ps[:, s],
                                kt_t[:, 2 * dp:2 * dp + 2,
                                     jcl * P:(jcl + 1) * P],
                                qt_sb[:, 2 * dp:2 * dp + 2,
                                      ih * IH:(ih + 1) * IH],
                                start=(dp == 0), stop=(dp == DP - 1),
                                perf_mode=DR)

                    def spair(t, ih):
                        ps = ps_mm[0].tile([P, 2, IH], F32, tag="st",
                                           name="s_ps")
                        yield lambda: s_half(ps, t, ih, 0)

                        def second():
                            s_half(ps, t, ih, 1)
                            tl = t % 2
                            nc.scalar.activation(
                                e_tiles[t // 2][:, 2 * tl:2 * tl + 2,
                                                ih * IH:(ih + 1) * IH],
                                ps[:], EXP, bias=0.0, scale=SCALE)
                        yield second

                    for tl in range(JT // P // 2):
                        t = jt * (JT // P // 2) + tl
                        for ih in ihs:
                            yield from spair(t, ih)

                def cs_ops(jt):
                    """denominator colsum accumulation for tile jt."""
                    for tl in range(JT // P // 2):
                        t = jt * (JT // P // 2) + tl
                        tloc = 2 * (t % 2)
                        for ih in range(NIH):
                            nc.tensor.matmul(
                                cs_pss[ih][:], cs_w[:],
                                e_tiles[t // 2][:, tloc:tloc + 2,
                                                ih * IH:(ih + 1) * IH],
                                start=(t == 0), stop=(t == NJP - 1),
                                perf_mode=DR)

                # ---- stage A: projection of jt overlapped with attention
                # of jt-1 and colsums of jt-2 (software pipeline) ----
                with tc.tile_pool(name="ps_mm", bufs=3,
                                  space="PSUM") as ps_mm_:
                    ps_mm.append(ps_mm_)
                    for op in qt_ops(0, xq_t=xq_t0):
                        op()
                    for jt in range(NJT):
                        cur = list(proj_ops(jt,
                                            xt_t=xt_t0 if jt == 0 else None))
                        if jt == 0:
                            # qt1 after the kt pairs: it waits on a later
                            # DMA and must not block ready kt work in the
                            # in-order PE queue
                            for op in (cur[0:4] + list(qt_ops(1))
                                       + cur[4:]):
                                op()
                            continue
                        pv = list(attn_ops(jt - 1)) if jt < NJT else []
                        if jt == NJT - 1:
                            pv += list(attn_ops(jt, ihs=(0,)))
                        n = max(len(cur), len(pv))
                        for i in range(n):
                            if i < len(cur):
                                cur[i]()
                            if i < len(pv):
                                pv[i]()
                        if jt >= 2:
                            cs_ops(jt - 2)
                    for op in attn_ops(NJT - 1, ihs=(1,)):
                        op()
                    cs_ops(NJT - 2)

            # ---- stage B: denominators, ctx, normalize + MLP ----
            with (
                tc.tile_pool(name="xtb", bufs=2) as mlp,
                tc.tile_pool(name="ps_ctx", bufs=3, space="PSUM") as psc,
                tc.tile_pool(name="ps_sc", bufs=1, space="PSUM") as pss,
                tc.tile_pool(name="ps_g", bufs=2, space="PSUM") as psg,
            ):
                if True:
                    def den_chain(ih):
                        recip_sb = mlp.tile([1, IH], F32, tag="recip")
                        nc.vector.reciprocal(recip_sb[:], cs_pss[ih][0:1])
                        r_sb = mlp.tile([P, IH], F32, tag="rsb")
                        nc.gpsimd.partition_broadcast(r_sb[:], recip_sb[:])
                        return r_sb

                    def ctx_block(ctx_ps, ih, dc):
                        for t in range(NJP):
                            tl = t % 2
                            nc.tensor.matmul(
                                ctx_ps[:],
                                v_sb[:, 2 * t:2 * t + 2,
                                     dc * P:(dc + 1) * P],
                                e_tiles[t // 2][:, 2 * tl:2 * tl + 2,
                                                ih * IH:(ih + 1) * IH],
                                start=(t == 0), stop=(t == NJP - 1),
                                perf_mode=DR)

                    def ctxn_op(ctxn, ctx_ps, r_sb, dc):
                        nc.vector.tensor_tensor(ctxn[:, dc], ctx_ps[:],
                                                r_sb[:],
                                                mybir.AluOpType.mult)

                    def g_dc(g_pss, ctxn, dc):
                        for mc_i in range(MC):
                            nc.tensor.matmul(g_pss[mc_i][:],
                                             w1_sb[:, dc, mc_i],
                                             ctxn[:, dc],
                                             start=(dc == 0),
                                             stop=(dc == DC - 1))

                    def mlp_end(g_pss, i0):
                        sc_ps = pss.tile([1, IH], F32, tag="sc")
                        h_sb = mlp.tile([P, MC, IH], BF16, tag="hsb")
                        # relu(g + b1) on two engines in parallel
                        nc.scalar.activation(
                            h_sb[:, 0], g_pss[0][:],
                            mybir.ActivationFunctionType.Relu,
                            bias=b1_sb[:, 0:1], scale=1.0)
                        nc.vector.tensor_scalar(
                            h_sb[:, 1], g_pss[1][:], b1_sb[:, 1:2], 0.0,
                            mybir.AluOpType.add, mybir.AluOpType.max)
                        for mc_i in range(MC):
                            nc.tensor.matmul(sc_ps[:],
                                             w2_sb[:, mc_i:mc_i + 1],
                                             h_sb[:, mc_i],
                                             start=(mc_i == 0),
                                             stop=(mc_i == MC - 1))
                        nc.scalar.add(out_sb[:, i0:i0 + IH], sc_ps[:],
                                      b2_sb[:])
                        nc.sync.dma_start(out[:, i0:i0 + IH],
                                          out_sb[:, i0:i0 + IH])

                    ctxn0 = mlp.tile([P, DC, IH], BF16, tag="ctxn")
                    ctxn1 = mlp.tile([P, DC, IH], BF16, tag="ctxn")
                    g0 = [psg.tile([P, IH], F32, tag="g", name="g_ps")
                          for _ in range(MC)]
                    r0_sb = r1_sb = None
                    for dc in range(DC):
                        cps = psc.tile([P, IH], F32, tag="ctx", name="c_ps")
                        ctx_block(cps, 0, dc)
                        if dc == 0:
                            # the last colsums + denominator chains hide
                            # behind this first 3.4us ctx block
                            cs_ops(NJT - 1)
                            r0_sb = den_chain(0)
                            r1_sb = den_chain(1)
                        ctxn_op(ctxn0, cps, r0_sb, dc)
                        if dc >= 2:
                            g_dc(g0, ctxn0, dc - 2)
                    cps = psc.tile([P, IH], F32, tag="ctx", name="c_ps")
                    ctx_block(cps, 1, 0)
                    g_dc(g0, ctxn0, 2)
                    g_dc(g0, ctxn0, 3)
                    mlp_end(g0, 0)
                    ctxn_op(ctxn1, cps, r1_sb, 0)
                    g1 = [psg.tile([P, IH], F32, tag="g", name="g_ps")
                          for _ in range(MC)]
                    for dc in range(1, DC):
                        cps = psc.tile([P, IH], F32, tag="ctx", name="c_ps")
                        ctx_block(cps, 1, dc)
                        ctxn_op(ctxn1, cps, r1_sb, dc)
                        g_dc(g1, ctxn1, dc - 1)
                    g_dc(g1, ctxn1, DC - 1)
                    mlp_end(g1, IH)

    nc.finalize()
    return nc


def _prep(inputs):
    """Host-side layout prep shared by all cores + per-core xq blocks."""
    f32 = np.float32
    bf16 = ml_dtypes.bfloat16
    fp8 = ml_dtypes.float8_e4m3
    x = np.ascontiguousarray(inputs["x"], dtype=f32)
    xt = np.ascontiguousarray(x.T)                                   # [IN, N]
    xt_r = np.ascontiguousarray(
        xt.reshape(KC, P, N).transpose(1, 0, 2).astype(fp8))         # [P, KC, N]

    def w_r(w):  # [IN, D] -> [P, KC, D], x8 scale into fp8 range
        return np.ascontiguousarray(
            (np.asarray(w, f32) * WS).reshape(KC, P, D)
            .transpose(1, 0, 2).astype(fp8))

    w1_r = np.ascontiguousarray(
        np.asarray(inputs["W1"], f32).reshape(DC, P, MC, P)
        .transpose(1, 0, 2, 3).astype(bf16))                         # [P, DC, MC, P]
    w2_r = np.ascontiguousarray(
        np.asarray(inputs["W2"], f32).reshape(MC, P).T.astype(bf16))  # [P, MC]

    def b_col(b, nchunks, scale=1.0):  # [nchunks*P] -> [P, nchunks]
        return np.ascontiguousarray(
            (np.asarray(b, f32) * scale).reshape(nchunks, P).T)

    # bv commutes out of attention: fold it into the MLP bias
    b1_eff = (np.asarray(inputs["b1"], f32)
              + np.asarray(inputs["bv"], f32) @ np.asarray(inputs["W1"], f32))

    shared = {
        "xt": xt_r,
        "wq": w_r(inputs["Wq"]),
        "wk": w_r(inputs["Wk"]),
        "wv": w_r(inputs["Wv"]),
        "w1": w1_r,
        "w2": w2_r,
        "bq": b_col(inputs["bq"], DC, WS),
        "b1": b_col(b1_eff, MC),
        "b2": np.asarray(inputs["b2"], f32).reshape(1, 1),
    }
    xqs = [np.ascontiguousarray(xt_r[:, :, c * NB:(c + 1) * NB])
           for c in range(NCORES)]
    return shared, xqs


def kernel(**inputs) -> np.ndarray:
    if "nc" not in _CACHE:
        _CACHE["nc"] = _build()
    nc = _CACHE["nc"]
    shared, xqs = _prep(inputs)
    in_maps = [dict(shared, xq=xqs[c]) for c in range(NCORES)]
    res = run_bass_kernel_spmd(nc, in_maps, core_ids=list(range(NCORES)))
    return np.concatenate([res.results[c]["out"][0] for c in range(NCORES)])
